# revision 19
# baseline (speedup 1.0000x reference)
"""Trainium2 Bass kernel for 2-layer GAT (nn_GAT_34832184770812).

Strategy (8 NeuronCores, dst-node sharded):
- Each core owns 1250 dst nodes; node ids are rotated per core so own nodes
  are local rows 0:1250 (keeps the SPMD program identical across cores).
- Phase A: T1 = features @ [W1 | W1@al1 | W1@ar1] (bf16, replicated) ->
  DRAM gather table T1tab[N, 384] (feat 256 | el 4 | ex-slot | pad) and
  er1 table [N, 4].
- Phase B (layer-1 edge phase): edges sorted by dst window (128 dst rows per
  window, padded to 128-edge chunks, chunk counts uniform across cores).
  Per 1024-edge superchunk: dma_gather of src rows; er per edge via a
  one-hot indicator matmul (Ind [dst,e] streamed from host); attention
  e = lrelu(el+er), ex = exp(e) (no max-subtraction needed: softmax is
  shift-invariant and exponents are O(1)); messages scaled by ex; segment
  sum over dst via indicator-transpose matmul (IndT resident in SBUF),
  with ex as extra columns producing softmax denominators in the same psum.
- Window finalize: normalize, ELU, transpose (PE), T2own = h @ W2p.
- AllGather T2own (bf16) -> T2all [N, 128] global gather table.
- Phase D (layer-2 edge phase): same structure, 1 head, 47 feats.
- log_softmax per window, output [1250, 47] f32 per core, host concat.
"""

import numpy as np
import ml_dtypes

BF16 = ml_dtypes.bfloat16

# problem constants (hardcoded per contract)
N = 10000
E = 320000
IN_FEATS = 256
H = 4
D = 64
HD = 256
OUTF = 47
NEG = 0.2
NCORES = 8
OWN = N // NCORES          # 1250
P = 128
NWIN = (OWN + P - 1) // P  # 10 windows (last has 98 nodes)
WIN_SIZES = [min(P, OWN - P * w) for w in range(NWIN)]
K = 8                      # chunks per superchunk
ROW1 = 384                 # T1 gather row (bf16): feat 256 | el 256:260 | ex 260:264 | pad
ROW2 = 128                 # T2 gather row (bf16): feat 0:47 | el2 47 | er2 48 | ex2 49 | pad
T2OWN_ROWS = NWIN * P      # 1280 (rows 1250:1280 zeroed)

_CACHE = {}


# ----------------------------------------------------------------------------
# host-side graph preprocessing
# ----------------------------------------------------------------------------

def _prep_graph(src, dst):
    """Per-core edge partition, window sort, uniform padding, one-hots."""
    src = np.asarray(src).astype(np.int64)
    dst = np.asarray(dst).astype(np.int64)
    core_of = dst // OWN
    per_core = []
    for c in range(NCORES):
        sel = np.nonzero(core_of == c)[0]
        dl = dst[sel] - OWN * c                       # local dst in [0, OWN)
        sl = (src[sel] - OWN * c) % N                 # local src
        order = np.argsort(dl, kind="stable")
        per_core.append((dl[order], sl[order], src[sel][order]))

    # uniform chunks per window across cores
    cw = []
    bounds = []
    for c in range(NCORES):
        dl = per_core[c][0]
        b = np.searchsorted(dl, [P * w for w in range(NWIN + 1)])
        bounds.append(b)
    for w in range(NWIN):
        mx = max(bounds[c][w + 1] - bounds[c][w] for c in range(NCORES))
        cw.append((int(mx) + P - 1) // P)
    nchunk = sum(cw)
    nsc = (nchunk + K - 1) // K
    pad_chunks = nsc * K - nchunk
    cw[-1] += pad_chunks
    nchunk = nsc * K

    chunk_win = []
    for w in range(NWIN):
        chunk_win += [w] * cw[w]

    ES = nchunk * P
    out = []
    for c in range(NCORES):
        dl, sl, sg = per_core[c]
        b = bounds[c]
        src_loc = np.zeros(ES, np.int16)
        src_glb = np.zeros(ES, np.int16)
        dstrow = np.full(ES, -1, np.int32)            # -1 = dummy
        pos = 0
        for w in range(NWIN):
            e0, e1 = b[w], b[w + 1]
            n = e1 - e0
            src_loc[pos : pos + n] = sl[e0:e1]
            src_glb[pos : pos + n] = sg[e0:e1]
            dstrow[pos : pos + n] = dl[e0:e1] - P * w
            pos += cw[w] * P
        # one-hot indicators, packed partition-major for batched loads:
        # ind[p=dstrow, ci*P + e]  (lhsT for er-broadcast matmul)
        # indt[p=e, ci*P + dstrow] (lhsT for segment-sum matmul)
        ind = np.zeros((P, ES), BF16)
        indt = np.zeros((P, ES), BF16)
        ch = np.arange(ES) // P
        e_in = np.arange(ES) % P
        real = dstrow >= 0
        r = np.nonzero(real)[0]
        ind[dstrow[r], ch[r] * P + e_in[r]] = 1
        indt[e_in[r], ch[r] * P + dstrow[r]] = 1
        # dma_gather wrapped idx layout per superchunk
        def wrap(ids):
            lay = np.zeros((nsc * P, K * P // 16), np.int16)
            for sc in range(nsc):
                blk = ids[sc * K * P : (sc + 1) * K * P]
                wr = np.zeros((16, K * P // 16), np.int16)
                kk = np.arange(K * P)
                wr[kk % 16, kk // 16] = blk
                lay[sc * P : (sc + 1) * P] = np.tile(wr, (8, 1))
            return lay
        # t2all row for global src s: row = c*1280 + local_r (single allgather
        # of the padded 1280-row local block)
        sgl = src_glb.astype(np.int64)
        t2row = (sgl // OWN) * T2OWN_ROWS + sgl % OWN
        out.append(dict(gidx1=wrap(src_loc), gidx2=wrap(t2row.astype(np.int16)), ind=ind, indt=indt))
    return out, cw, nchunk, nsc, chunk_win


# ----------------------------------------------------------------------------
# program build
# ----------------------------------------------------------------------------

def build_program(nchunk, nsc, chunk_win, reps=1):
    import concourse.tile as tile
    from concourse import bacc, mybir
    from concourse.masks import make_identity

    NT = (N + P - 1) // P                              # 79 node tiles
    win_first = {}
    win_last = {}
    for ci, w in enumerate(chunk_win):
        if w not in win_first:
            win_first[w] = ci
        win_last[w] = ci


    nc = bacc.Bacc("TRN2", target_bir_lowering=False, debug=False, num_devices=NCORES)
    dt = mybir.dt
    featT = nc.declare_dram_parameter("featT", [IN_FEATS, N], dt.bfloat16, isOutput=False)
    W1p = nc.declare_dram_parameter("W1p", [IN_FEATS, 264], dt.bfloat16, isOutput=False)
    W2p = nc.declare_dram_parameter("W2p", [HD, 52], dt.bfloat16, isOutput=False)
    gidx1 = nc.declare_dram_parameter("gidx1", [nsc * P, K * P // 16], dt.int16, isOutput=False)
    gidx2 = nc.declare_dram_parameter("gidx2", [nsc * P, K * P // 16], dt.int16, isOutput=False)
    indp = nc.declare_dram_parameter("ind", [P, nchunk * P], dt.bfloat16, isOutput=False)
    indtp = nc.declare_dram_parameter("indt", [P, nchunk * P], dt.bfloat16, isOutput=False)
    outp = nc.declare_dram_parameter("out", [OWN, OUTF], dt.float32, isOutput=True)

    t1tab = nc.dram_tensor("t1tab", [NT * P, ROW1], dt.bfloat16)

    IC = K * P // 16                                   # idx cols per superchunk

    with tile.TileContext(nc) as tc:
        with (
            tc.tile_pool(name="const", bufs=1) as constp,
            tc.tile_pool(name="res", bufs=1) as respool,
            tc.tile_pool(name="dram", bufs=1, space="DRAM") as dramp,
        ):
            ident = constp.tile([P, P], dt.float32)
            make_identity(nc, ident[:])

            indt_res = respool.tile([P, nchunk * P], dt.bfloat16, tag="res")
            t2bounce = dramp.tile([T2OWN_ROWS, ROW2], dt.bfloat16, tag="t2b")
            t2all = dramp.tile([NCORES * T2OWN_ROWS, ROW2], dt.bfloat16)

            # resident idx tables (tiny)
            gixt1 = constp.tile([P, nsc, IC], dt.int16)
            nc.sync.dma_start(out=gixt1[:], in_=gidx1[:, :].rearrange("(s p) c -> p s c", p=P))
            gixt2 = constp.tile([P, nsc, IC], dt.int16)
            nc.sync.dma_start(out=gixt2[:], in_=gidx2[:, :].rearrange("(s p) c -> p s c", p=P))

            # resident attention tables
            er_all = constp.tile([P, NWIN, 4], dt.bfloat16)
            er2_all = constp.tile([P, NWIN, 1], dt.bfloat16)
            nc.vector.memset(er2_all[:], 0)
            logit_all = constp.tile([P, NWIN, OUTF], dt.float32)

            for r in range(reps):
                last = r == reps - 1
                # ---------------- phase A: T1 table ----------------
                with (
                    tc.tile_pool(name="pa", bufs=4) as pa,
                    tc.tile_pool(name="paps", bufs=4, space="PSUM") as paps,
                    tc.tile_pool(name="w1pool", bufs=1) as w1pool,
                ):
                    w1t = w1pool.tile([P, 264], dt.bfloat16, tag="w1a")
                    nc.sync.dma_start(out=w1t[:], in_=W1p[0:P, :])
                    w1b = w1pool.tile([P, 264], dt.bfloat16, tag="w1b")
                    nc.sync.dma_start(out=w1b[:], in_=W1p[P:IN_FEATS, :])
                    BLK = 8                                        # node tiles per block
                    for b0 in range(0, NT, BLK):
                        nb = min(BLK, NT - b0)
                        mm = min(nb * P, N - b0 * P)               # real nodes in block
                        lt0 = pa.tile([P, BLK * P], dt.bfloat16, tag="lt")
                        nc.sync.dma_start(out=lt0[:, 0:mm], in_=featT[0:P, b0 * P : b0 * P + mm])
                        lt1 = pa.tile([P, BLK * P], dt.bfloat16, tag="lt2")
                        nc.sync.dma_start(out=lt1[:, 0:mm], in_=featT[P : 2 * P, b0 * P : b0 * P + mm])
                        rows = pa.tile([P, BLK, 264], dt.bfloat16, tag="row")
                        for j in range(nb):
                            m = min(P, mm - j * P)
                            ps = paps.tile([P, 264], dt.float32, space="PSUM", tag="paps")
                            nc.tensor.matmul(ps[0:m, :], lhsT=lt0[:, j * P : j * P + m], rhs=w1t[:], start=True, stop=False)
                            nc.tensor.matmul(ps[0:m, :], lhsT=lt1[:, j * P : j * P + m], rhs=w1b[:], start=False, stop=True)
                            if j % 2 == 0:
                                nc.vector.tensor_copy(rows[:, j, :], ps[:, :])
                            else:
                                nc.scalar.activation(rows[:, j, :], ps[:, :], mybir.ActivationFunctionType.Copy)
                            w = b0 + j
                            if w < NWIN:
                                # own-node windows: keep er resident
                                nc.vector.tensor_copy(er_all[:, w, :], rows[:, j, 260:264])
                        nc.scalar.dma_start(
                            out=t1tab[b0 * P : (b0 + nb) * P, 0:260].rearrange("(j p) f -> p j f", p=P),
                            in_=rows[:, 0:nb, 0:260],
                        )

                # resident IndT load (issued after phase A loads on sync queue;
                # overlaps phase B, per-block deps)
                nload = 4
                if r == 0:
                    step = (nchunk + nload - 1) // nload * P
                    for i in range(nload):
                        lo = i * step
                        hi = min(nchunk * P, lo + step)
                        if lo < hi:
                            nc.sync.dma_start(out=indt_res[:, lo:hi], in_=indtp[:, lo:hi])
                indt_tiles = [indt_res[:, ci * P : (ci + 1) * P] for ci in range(nchunk)]

                # ---------------- phase B: layer-1 edge phase ----------------
                with (
                    tc.tile_pool(name="pg", bufs=6) as pg,
                    tc.tile_pool(name="pb", bufs=3) as pb,
                    tc.tile_pool(name="pbfin", bufs=2) as pbfin,
                    tc.tile_pool(name="wps", bufs=4, space="PSUM") as wps,
                    tc.tile_pool(name="erps", bufs=2, space="PSUM") as erps,
                    tc.tile_pool(name="finps", bufs=2, space="PSUM") as finps,
                    tc.tile_pool(name="w2pool", bufs=1) as w2pool,
                ):
                    w2t = w2pool.tile([P, 52], dt.bfloat16, tag="w2a")
                    nc.sync.dma_start(out=w2t[:], in_=W2p[0:P, :])
                    w2b = w2pool.tile([P, 52], dt.bfloat16, tag="w2b")
                    nc.sync.dma_start(out=w2b[:], in_=W2p[P:HD, :])

                    win_psum = None
                    for sc in range(nsc):
                        g = pg.tile([P, K, ROW1], dt.bfloat16, tag="g")
                        nc.gpsimd.dma_gather(g[:, :, :], t1tab[:, :], gixt1[:, sc, :], K * P, K * P, ROW1)
                        ind_sc = pb.tile([P, K * P], dt.bfloat16, tag="ind")
                        nc.scalar.dma_start(out=ind_sc[:], in_=indp[:, sc * K * P : (sc + 1) * K * P])
                        er_psum = erps.tile([P, K * 4], dt.float32, space="PSUM", tag="erp")
                        for j in range(K):
                            ci = sc * K + j
                            w = chunk_win[ci]
                            nc.tensor.matmul(
                                er_psum[:, j * 4 : (j + 1) * 4],
                                lhsT=ind_sc[:, j * P : (j + 1) * P], rhs=er_all[:, w, :],
                                start=True, stop=True,
                            )
                        att = pb.tile([P, K, 4], dt.float32, tag="att")
                        nc.vector.tensor_tensor(
                            out=att[:], in0=g[:, :, 256:260],
                            in1=er_psum[:].rearrange("p (c h) -> p c h", c=K),
                            op=mybir.AluOpType.add,
                        )
                        # exp(lrelu(x)) = max(exp(x), exp(0.2 x)); Exp table only
                        e1 = pb.tile([P, K, 4], dt.float32, tag="e1")
                        nc.scalar.activation(e1[:], att[:], mybir.ActivationFunctionType.Exp)
                        e2 = pb.tile([P, K, 4], dt.float32, tag="e2")
                        nc.scalar.activation(e2[:], att[:], mybir.ActivationFunctionType.Exp, scale=NEG)
                        nc.vector.tensor_tensor(out=g[:, :, 260:264], in0=e1[:], in1=e2[:], op=mybir.AluOpType.max)
                        nc.vector.tensor_tensor(
                            out=g[:, :, 0:HD].rearrange("p c (h d) -> p c h d", h=H),
                            in0=g[:, :, 0:HD].rearrange("p c (h d) -> p c h d", h=H),
                            in1=g[:, :, 260:264, None].broadcast_to([P, K, 4, D]),
                            op=mybir.AluOpType.mult,
                        )
                        for j in range(K):
                            ci = sc * K + j
                            w = chunk_win[ci]
                            if ci == win_first[w]:
                                win_psum = wps.tile([P, 264], dt.float32, space="PSUM", tag="acc")
                            nc.tensor.matmul(
                                win_psum[:],
                                lhsT=indt_tiles[ci],
                                rhs=g[:, j, 0:264],
                                start=(ci == win_first[w]),
                                stop=(ci == win_last[w]),
                            )
                            if ci == win_last[w]:
                                m = WIN_SIZES[w]
                                den = pbfin.tile([P, 4], dt.float32, tag="den")
                                nc.vector.tensor_scalar_max(den[:], win_psum[:, 260:264], 1e-9)
                                rec = pbfin.tile([P, 4], dt.float32, tag="rec")
                                nc.vector.reciprocal(rec[:], den[:])
                                h_sb = pbfin.tile([P, HD], dt.float32, tag="hsb")
                                nc.vector.tensor_tensor(
                                    out=h_sb[:].rearrange("p (h d) -> p h d", h=H),
                                    in0=win_psum[:, 0:HD].rearrange("p (h d) -> p h d", h=H),
                                    in1=rec[:, :, None].broadcast_to([P, H, D]),
                                    op=mybir.AluOpType.mult,
                                )
                                # ELU: relu(h) + exp(min(h,0)) - 1
                                hneg = pbfin.tile([P, HD], dt.float32, tag="hneg")
                                nc.vector.tensor_scalar_min(hneg[:], h_sb[:], 0.0)
                                hexp = pbfin.tile([P, HD], dt.float32, tag="hexp")
                                nc.scalar.activation(hexp[:], hneg[:], mybir.ActivationFunctionType.Exp)
                                nc.vector.tensor_scalar_max(h_sb[:], h_sb[:], 0.0)
                                nc.vector.tensor_tensor(out=h_sb[:], in0=h_sb[:], in1=hexp[:], op=mybir.AluOpType.add)
                                nc.vector.tensor_scalar_add(h_sb[:], h_sb[:], -1.0)
                                # transpose h (2x PE) -> hT bf16
                                hT = pbfin.tile([P, 2, P], dt.bfloat16, tag="hT")
                                for half in range(2):
                                    tp = finps.tile([P, P], dt.float32, space="PSUM", tag="fin")
                                    nc.tensor.transpose(out=tp[:, 0:m], in_=h_sb[0:m, half * P : (half + 1) * P], identity=ident[0:m, 0:m])
                                    nc.vector.tensor_copy(hT[:, half, 0:m], tp[:, 0:m])
                                # T2 rows = h @ W2p, direct to bounce + resident er2
                                t2ps = finps.tile([P, 52], dt.float32, space="PSUM", tag="fin")
                                nc.tensor.matmul(t2ps[0:m, :], lhsT=hT[:, 0, 0:m], rhs=w2t[:], start=True, stop=False)
                                nc.tensor.matmul(t2ps[0:m, :], lhsT=hT[:, 1, 0:m], rhs=w2b[:], start=False, stop=True)
                                t2row = pbfin.tile([P, 52], dt.bfloat16, tag="t2row")
                                nc.vector.tensor_copy(t2row[0:m, :], t2ps[0:m, :])
                                nc.vector.tensor_copy(er2_all[0:m, w, :], t2row[0:m, 48:49])
                                nc.sync.dma_start(out=t2bounce[w * P : w * P + m, 0:52], in_=t2row[0:m, :])


                nc.gpsimd.collective_compute(
                    "AllGather",
                    mybir.AluOpType.bypass,
                    replica_groups=[list(range(NCORES))],
                    ins=[t2bounce[:, :].opt()],
                    outs=[t2all[:, :].opt()],
                )

                # ---------------- phase D: layer-2 edge phase ----------------
                with (
                    tc.tile_pool(name="pg2", bufs=6) as pg2,
                    tc.tile_pool(name="pd", bufs=3) as pd,
                    tc.tile_pool(name="pdfin", bufs=2) as pdfin,
                    tc.tile_pool(name="wps2", bufs=4, space="PSUM") as wps2,
                    tc.tile_pool(name="erps2", bufs=2, space="PSUM") as erps2,
                ):
                    win_psum2 = None
                    for sc in range(nsc):
                        g2 = pg2.tile([P, K, ROW2], dt.bfloat16, tag="g2")
                        nc.gpsimd.dma_gather(g2[:, :, :], t2all[:, :], gixt2[:, sc, :], K * P, K * P, ROW2)
                        ind_sc = pd.tile([P, K * P], dt.bfloat16, tag="ind2")
                        nc.scalar.dma_start(out=ind_sc[:], in_=indp[:, sc * K * P : (sc + 1) * K * P])
                        er_psum2 = erps2.tile([P, K], dt.float32, space="PSUM", tag="erp2")
                        for j in range(K):
                            ci = sc * K + j
                            w = chunk_win[ci]
                            nc.tensor.matmul(
                                er_psum2[:, j : j + 1],
                                lhsT=ind_sc[:, j * P : (j + 1) * P], rhs=er2_all[:, w, :],
                                start=True, stop=True,
                            )
                        att = pd.tile([P, K], dt.float32, tag="attl2")
                        nc.vector.tensor_tensor(
                            out=att[:, :, None], in0=g2[:, :, 47:48], in1=er_psum2[:, :, None],
                            op=mybir.AluOpType.add,
                        )
                        e1 = pd.tile([P, K], dt.float32, tag="e1b")
                        nc.scalar.activation(e1[:], att[:], mybir.ActivationFunctionType.Exp)
                        e2 = pd.tile([P, K], dt.float32, tag="e2b")
                        nc.scalar.activation(e2[:], att[:], mybir.ActivationFunctionType.Exp, scale=NEG)
                        nc.vector.tensor_tensor(out=g2[:, :, 49:50], in0=e1[:, :, None], in1=e2[:, :, None], op=mybir.AluOpType.max)
                        nc.vector.tensor_tensor(
                            out=g2[:, :, 0:48],
                            in0=g2[:, :, 0:48],
                            in1=g2[:, :, 49:50].broadcast_to([P, K, 48]),
                            op=mybir.AluOpType.mult,
                        )
                        for j in range(K):
                            ci = sc * K + j
                            w = chunk_win[ci]
                            if ci == win_first[w]:
                                win_psum2 = wps2.tile([P, 50], dt.float32, space="PSUM", tag="acc2")
                            nc.tensor.matmul(
                                win_psum2[:],
                                lhsT=indt_tiles[ci],
                                rhs=g2[:, j, 0:50],
                                start=(ci == win_first[w]),
                                stop=(ci == win_last[w]),
                            )
                            if ci == win_last[w]:
                                den = pdfin.tile([P, 1], dt.float32, tag="den2")
                                nc.vector.tensor_scalar_max(den[:], win_psum2[:, 49:50], 1e-9)
                                rec = pdfin.tile([P, 1], dt.float32, tag="rec2")
                                nc.vector.reciprocal(rec[:], den[:])
                                nc.vector.tensor_scalar(
                                    out=logit_all[:, w, :], in0=win_psum2[:, 0:OUTF],
                                    scalar1=rec[:, 0:1], scalar2=None,
                                    op0=mybir.AluOpType.mult,
                                )
                    # deferred log_softmax over all windows
                    if last:
                        mx = pdfin.tile([P, NWIN, 1], dt.float32, tag="mx")
                        nc.vector.tensor_reduce(mx[:], logit_all[:], mybir.AxisListType.X, mybir.AluOpType.max)
                        nc.vector.tensor_tensor(
                            out=logit_all[:], in0=logit_all[:],
                            in1=mx[:].broadcast_to([P, NWIN, OUTF]),
                            op=mybir.AluOpType.subtract,
                        )
                        exps = pdfin.tile([P, NWIN, OUTF], dt.float32, tag="exps")
                        nc.scalar.activation(exps[:], logit_all[:], mybir.ActivationFunctionType.Exp)
                        se = pdfin.tile([P, NWIN, 1], dt.float32, tag="se")
                        nc.vector.tensor_reduce(se[:], exps[:], mybir.AxisListType.X, mybir.AluOpType.add)
                        lse = pdfin.tile([P, NWIN, 1], dt.float32, tag="lse")
                        nc.scalar.activation(lse[:], se[:], mybir.ActivationFunctionType.Ln)
                        nc.vector.tensor_tensor(
                            out=logit_all[:], in0=logit_all[:],
                            in1=lse[:].broadcast_to([P, NWIN, OUTF]),
                            op=mybir.AluOpType.subtract,
                        )
                        nc.sync.dma_start(
                            out=outp[0 : (NWIN - 1) * P, :].rearrange("(w p) f -> p w f", p=P),
                            in_=logit_all[:, 0 : NWIN - 1, :],
                        )
                        mlast = OWN - (NWIN - 1) * P
                        nc.sync.dma_start(
                            out=outp[(NWIN - 1) * P : OWN, :],
                            in_=logit_all[0:mlast, NWIN - 1, :],
                        )
    nc.compile()
    return nc


# ----------------------------------------------------------------------------
# host entry
# ----------------------------------------------------------------------------

def _host_inputs(features, src, dst, W1, al1, ar1, W2, al2, ar2):
    feats = np.asarray(features, np.float32)
    W1 = np.asarray(W1, np.float32)
    W2 = np.asarray(W2, np.float32)
    al1 = np.asarray(al1, np.float32)
    ar1 = np.asarray(ar1, np.float32)
    al2 = np.asarray(al2, np.float32)
    ar2 = np.asarray(ar2, np.float32)

    Wl1 = np.stack([W1[:, h * D : (h + 1) * D] @ al1[h] for h in range(H)], axis=1)
    Wr1 = np.stack([W1[:, h * D : (h + 1) * D] @ ar1[h] for h in range(H)], axis=1)
    W1p = np.concatenate([W1, Wl1, Wr1], axis=1).astype(BF16)          # [256, 264]
    Wl2 = (W2 @ al2[0])[:, None]
    Wr2 = (W2 @ ar2[0])[:, None]
    W2p = np.concatenate([W2, Wl2, Wr2, np.zeros((HD, 3), np.float32)], axis=1).astype(BF16)  # [256, 52]

    graph, cw, nchunk, nsc, chunk_win = _prep_graph(src, dst)
    featT = np.ascontiguousarray(feats.T)                               # [256, N]
    in_maps = []
    for c in range(NCORES):
        featTl = np.roll(featT, -OWN * c, axis=1)                       # local node order
        in_maps.append(dict(
            featT=featTl.astype(BF16),
            W1p=W1p, W2p=W2p,
            gidx1=graph[c]["gidx1"], gidx2=graph[c]["gidx2"],
            ind=graph[c]["ind"], indt=graph[c]["indt"],
        ))
    return in_maps, nchunk, nsc, chunk_win


def kernel(features, src, dst, W1, al1, ar1, W2, al2, ar2):
    from concourse.bass_utils import run_bass_kernel_spmd

    in_maps, nchunk, nsc, chunk_win = _host_inputs(
        features, src, dst, W1, al1, ar1, W2, al2, ar2)
    key = (nchunk, nsc, tuple(chunk_win))
    if key not in _CACHE:
        _CACHE[key] = build_program(nchunk, nsc, chunk_win, reps=1)
    nc = _CACHE[key]
    res = run_bass_kernel_spmd(nc, in_maps, core_ids=list(range(NCORES)))
    return np.concatenate([res.results[c]["out"] for c in range(NCORES)], axis=0)

